# revision 3
# baseline (speedup 1.0000x reference)
# Trainium2 Bass kernel for nn_Block_SA (dense_cnn self-attention block), v2.
#
# Per-sample computation (C=64 channels, 64x64 spatial, N=4096 positions):
#   v   = relu(bn1(conv1x1(x)))                      # V for attention
#   s   = (x^T x) / sqrt(C)                          # [N, N] scores, Q=K=x
#   p   = softmax(s, axis=-1)
#   a   = V p^T
#   z   = relu(bn2(depthwise3x3(a)))
#   out = bn3(conv1x1(z)) + x
#
# Distribution: batch B=8, one sample per NeuronCore (data parallel).
#
# v2 design (PE effectively runs ~1.2 GHz on this part; cut PE cycles + split
# exp across ACT and DVE):
#   - scores: f32r, row-packed 2-at-a-time via tile_position (as v1).
#   - exp: shifted by -4.5 (softmax shift-invariant) so unnormalized p fits
#     fp8e5's range. Most groups on ACT (exp -> fp8e5 direct); a few groups
#     per chunk on DVE via a Schraudolph-style bit trick: one tensor_scalar
#     (mult, add) with int8 output whose bits ARE the fp8e5 value, biased
#     +12 bits (x8) to stay in [0,127]; compensated by a 1/8-prescaled vt.
#   - AV: fp8 DoubleRow matmuls (0.5 cyc/row): lhsT = vt pairs, rhs = exp
#     output pairs [128, 2, 512]. vt blocks are 68 wide (col 64 = ones for
#     the denominator, 65:68 zero padding); pair stride must be 4-byte
#     aligned (ISA dual-fp8 LDWEIGHTS restriction).
#   - V^T: bf16 matmuls (f32 mode is 4 cyc/row, f32r is 4x penalized under
#     256 free), bias via ones row of a bf16 x copy; DVE relu writes both
#     vt8 (fp8e4) and vt8s (fp8e4, x0.125 for the DVE-exp groups).
#   - softmax denominator: po row 64 (ones column of vt). 1/den via DMA
#     scatter [1,512]->[4,128], DVE reciprocal (fp16), and 4 K=1 fp16
#     broadcast matmuls.
#   - depthwise 3x3: fp8 DoubleRow tap pairs over a padded-pitch-66 fp8 copy
#     of y (zero columns between rows and zero guard rows), taps paired by
#     (byte-stride % 4 == 0); single taps use a zeros second k-tile.
#   - conv3 + bias via zr ones row; residual add; DMA out. (as v1)

import numpy as np

_EPS = 1e-5
_C = 64
_CP1 = 65
_N = 4096
_CH = 512
_MT = 128          # m-tile size
_NMT = _N // _MT   # 32 m-tiles
_NG = 16           # groups of 2 m-tiles per chunk
_NCH = _N // _CH   # 8 chunks
_W = 64

_BIAS = -4.5                      # exp shift (softmax-invariant)
_A8 = 4.0 / np.log(2.0)           # e5m2 bits per nat
_B8 = 59.75 + 12.0                # +12 bits = x8 (DVE converts round-to-nearest)
_TS1 = float(_A8 * 0.125)         # tensor_scalar mult
_TS2 = float(_B8 + _A8 * _BIAS)   # tensor_scalar add

_DVE_N = 0         # exp groups per chunk on DVE (rest on ACT)
_DVE_N0 = 0        # fewer on chunk 0 (DVE busy building vt)

# vt layout: blocks of 128 cols (64 V + ones col + zero pad), m-tile t at
# offset 128*t; DoubleRow k-tile stride 128 (dual-fp8 LDWEIGHTS requires the
# k-tile byte stride to be a multiple of 64 -- probed empirically).
_VTW = 68                         # used cols per block (out partitions)
_VTB = 128                        # block pitch
_VTCOLS = _NMT * _VTB

# depthwise pairing (row pitch 128): tap k=(3dy+dx) offset o_k =
# 128*(dy-1)+(dx-1); pairs need (o_b - o_a) % 64 == 0; singles use a zeros
# 2nd k-tile (rhs k-stride 0).
_DW_PITCH = 128
_DW_PAIRS = [(0, 3), (2, 5), (1, 7)]   # strides +128, +128, +256
_DW_SINGLES = [4, 6, 8]
_DW_INSTR = _DW_PAIRS + [(k, None) for k in _DW_SINGLES]

_NCONST = 64 + 64 + 1 + 9 * 64   # w1aug | w3aug | b2p | dw diags
_NCONST8 = 9 * 64            # 9 dw diag blocks [64, 64] fp8e4

_STATE = {}


def _dw_off(k):
    dy, dx = k // 3 - 1, k % 3 - 1
    return _DW_PITCH * dy + dx


def _build_program():
    import concourse.bacc as bacc
    import concourse.tile as tile
    from concourse import mybir
    from concourse.bass_types import AP

    F32 = mybir.dt.float32
    F32R = mybir.dt.float32r
    F16 = mybir.dt.float16
    BF16 = mybir.dt.bfloat16
    E4 = mybir.dt.float8e4
    E5 = mybir.dt.float8e5
    I8 = mybir.dt.int8
    U32 = mybir.dt.uint32
    AF = mybir.ActivationFunctionType
    ALU = mybir.AluOpType
    DR = mybir.MatmulPerfMode.DoubleRow
    ONE_BITS = 0x3F800000

    nc = bacc.Bacc(None)

    xd = nc.dram_tensor("x", [_C, _N], F32, kind="ExternalInput")
    cd = nc.dram_tensor("consts", [_CP1, _NCONST], F32, kind="ExternalInput")
    c8d = nc.dram_tensor("consts8", [_C, _NCONST8], E4, kind="ExternalInput")
    outd = nc.dram_tensor("out", [_C, _N], F32, kind="ExternalOutput")

    with tile.TileContext(nc) as tc:
        with (
            tc.tile_pool(name="persist", bufs=1) as pp,
            tc.tile_pool(name="small", bufs=2) as sp,
            tc.tile_pool(name="pt8_pool", bufs=3) as p8p,
            tc.tile_pool(name="pt8d_pool", bufs=2) as p8dp,
            tc.tile_pool(name="ps_pool", bufs=2, space="PSUM") as psp,
            tc.tile_pool(name="po_pool", bufs=1, space="PSUM") as pop,
            tc.tile_pool(name="aux_pool", bufs=3, space="PSUM") as auxp,
        ):
            # ---- input staging: x duplicated on partitions 64:128 for the
            # row-packed score matmuls; f32r copy + bf16 copy (V path).
            xs2 = pp.tile([_MT, _N], F32, name="xs2", tag="xs2")
            xr2 = pp.tile([_MT, _N], F32R, name="xr2", tag="xr2")
            xab = pp.tile([_CP1, _N], BF16, name="xab", tag="xab")
            for s in range(4):
                sl = slice(1024 * s, 1024 * (s + 1))
                nc.sync.dma_start(xs2[0:_C, sl], xd[:, sl])
                nc.sync.dma_start(xs2[_C:_MT, sl], xd[:, sl])
                nc.vector.tensor_copy(xr2[:, sl], xs2[:, sl])
                nc.vector.tensor_copy(xab[0:_C, sl], xs2[0:_C, sl])
            nc.gpsimd.memset(xab[_C:_CP1, :], 1.0)

            cs = pp.tile([_CP1, _NCONST], F32, name="cs", tag="cs")
            nc.scalar.dma_start(cs[:], cd[:])
            c8s = pp.tile([_C, _NCONST8], E4, name="c8s", tag="c8s")
            nc.scalar.dma_start(c8s[:], c8d[:])
            b2s = cs[0:_C, 128:129]

            w1b = pp.tile([_CP1, _C], BF16, name="w1b", tag="w1b")
            nc.vector.tensor_copy(w1b[:], cs[:, 0:64])
            w3r = pp.tile([_CP1, _C], F32R, name="w3r", tag="w3r")
            nc.vector.tensor_copy(w3r[:], cs[:, 64:128])
            dgr = pp.tile([_C, 9 * _C], F32R, name="dgr", tag="dgr")
            nc.vector.tensor_copy(dgr[:], cs[0:_C, 129 : 129 + 9 * _C])

            # broadcast helpers: den quarters at partitions 0/32/64/96 of a
            # 128-partition tile (1.0 elsewhere); reciprocals are then moved
            # into diagonal blocks of inv4z [4, 512] (zeros elsewhere) so ONE
            # K=4 fp16 matmul at position (0,0) broadcasts 1/den.
            ones16 = pp.tile([4, _C], F16, name="ones16", tag="ones16")
            nc.vector.memset(ones16[:], 1.0)
            den97 = pp.tile([_MT, _MT], F32, name="den97", tag="den97")
            nc.gpsimd.memset(den97[:], 1.0)
            inv97 = pp.tile([_MT, _MT], F16, name="inv97", tag="inv97")
            inv4z = pp.tile([4, _CH], F16, name="inv4z", tag="inv4z")
            nc.vector.memset(inv4z[:], 0.0)
            ebias = pp.tile([_MT, 1], F32, name="ebias", tag="ebias")
            nc.vector.memset(ebias[:], _BIAS)

            # vt blocks (bf16): m-tile t at col 128*t; col 64 = ones.
            vt8 = pp.tile([_MT, _VTCOLS], BF16, name="vt8", tag="vt8")

            def vt_block(tile8, t):
                off = _VTB * t
                return tile8[:, off : off + _VTW]

            def vt_pair_ap(tile8, g):
                b = vt_block(tile8, 2 * g)
                return AP(tensor=b.tensor, offset=b.offset,
                          ap=[list(b.ap[0]), [_VTB, 2], [1, _VTW]])

            # y (attention out, normalized) in fp8e4, pitch-66 padded layout
            # with zero guard rows: row h data at 66*(h+1)+1 .. +65.
            yr8 = pp.tile([_C, _DW_PITCH * 66], F32R, name="yr8", tag="yr8")
            # (row h data at 128*(h+1)+1 .. +65; everything else stays zero)
            nc.gpsimd.memset(yr8[:].bitcast(U32), 0)

            def y_rows(h0, nh, extra_off=0):
                """2D AP [(66, nh), (1, 64)] starting at out-row h0 (+extra)."""
                off = _DW_PITCH * (h0 + 1) + 1 + extra_off
                base = yr8[:, off : off + 1]
                return AP(tensor=base.tensor, offset=base.offset,
                          ap=[list(base.ap[0]), [_DW_PITCH, nh], [1, _W]])

            def y_rows_pair(h0, nh, offa, offb):
                off = _DW_PITCH * (h0 + 1) + 1 + offa
                base = yr8[:, off : off + 1]
                return AP(tensor=base.tensor, offset=base.offset,
                          ap=[list(base.ap[0]), [offb - offa, 2],
                              [_DW_PITCH, nh], [1, _W]])

            # post-depthwise activations (+ones row) feeding conv3
            zr = pp.tile([_CP1, _N], F32R, name="zr", tag="zr")
            nc.gpsimd.memset(zr[_C:_CP1, :].bitcast(U32), ONE_BITS)
            zrv = zr[0:_C, :].rearrange("c (h w) -> c h w", w=_W)

            # ---- V^T groups: emitted lazily; bf16 matmuls + two DVE
            # relu writes (vt8 and the 1/8-scaled vt8s).
            _vt_emitted = [0]

            def emit_vt_groups(need_mtiles):
                while _vt_emitted[0] * 4 < need_mtiles:
                    vg = _vt_emitted[0]
                    vps = auxp.tile([_MT, 4 * _C], F32, name="vps", tag="aux")
                    for j in range(4):
                        t = 4 * vg + j
                        nc.tensor.matmul(
                            vps[:, _C * j : _C * (j + 1)],
                            lhsT=xab[:, _MT * t : _MT * (t + 1)],
                            rhs=w1b[:],
                            start=True, stop=True,
                        )
                    for j in range(4):
                        t = 4 * vg + j
                        nc.vector.tensor_relu(vt_block(vt8, t)[:, 0:_C],
                                              vps[:, _C * j : _C * (j + 1)])
                        nc.gpsimd.memset(vt_block(vt8, t)[:, _C : _C + 1], 1.0)
                    _vt_emitted[0] += 1

            # ---- depthwise via fp8 DoubleRow tap pairs ----
            def emit_dw(h0, h1):
                nh = h1 - h0
                dwp = auxp.tile([_C, nh * _W], F32, name="dwp", tag="aux")
                for k in range(9):
                    nc.tensor.matmul(
                        dwp[:], lhsT=dgr[:, _C * k : _C * (k + 1)],
                        rhs=y_rows(h0, nh, _dw_off(k)),
                        start=(k == 0), stop=(k == 8),
                        skip_group_check=True,
                    )
                nc.vector.tensor_scalar(
                    zrv[:, h0:h1, :],
                    dwp.rearrange("c (h w) -> c h w", w=_W),
                    b2s, 0.0, op0=ALU.add, op1=ALU.max,
                )

            def emit_conv3(c):
                pc = auxp.tile([_C, _CH], F32, name="pc", tag="aux")
                nc.tensor.matmul(
                    pc[:], lhsT=w3r[:], rhs=zr[:, _CH * c : _CH * (c + 1)],
                    start=True, stop=True,
                )
                outt = sp.tile([_C, _CH], F32, name="outt", tag="outt", bufs=2)
                nc.vector.tensor_tensor(
                    outt[:], pc[:], xs2[0:_C, _CH * c : _CH * (c + 1)],
                    op=ALU.add,
                )
                nc.sync.dma_start(outd[:, _CH * c : _CH * (c + 1)], outt[:])

            # ---- tail of chunk ci: normalize y, then dw rows ----
            def emit_normalize(ci, po):
                usb = sp.tile([_CP1, _CH], F32, name="usb", tag="usb", bufs=2)
                nc.vector.tensor_copy(usb[:], po[0:_CP1, :])
                for j in range(4):
                    nc.sync.dma_start(den97[32 * j : 32 * j + 1, :],
                                      usb[_C : _C + 1, _MT * j : _MT * (j + 1)])
                with nc.allow_low_precision(reason="fp16 invden"):
                    nc.vector.reciprocal(inv97[:], den97[:])
                for j in range(4):
                    nc.sync.dma_start(
                        inv4z[j : j + 1, _MT * j : _MT * (j + 1)],
                        inv97[32 * j : 32 * j + 1, :],
                    )
                return usb, inv4z

            def emit_normalize2(ci, usb, inv4):
                bcp = auxp.tile([_C, _CH], F32, name="bcp", tag="aux")
                nc.tensor.matmul(
                    bcp[:], lhsT=ones16[:], rhs=inv4[:],
                    start=True, stop=True,
                )
                # y rows 8ci..8ci+8 (2D strided dst into the padded layout)
                dst = AP(
                    tensor=yr8.tensor,
                    offset=yr8[:, _DW_PITCH * (8 * ci + 1) + 1:].offset,
                    ap=[list(yr8[:].ap[0]), [_DW_PITCH, 8], [1, _W]],
                )
                nc.vector.tensor_tensor(
                    dst,
                    usb[0:_C, :].rearrange("c (h w) -> c h w", w=_W),
                    bcp[:].rearrange("c (h w) -> c h w", w=_W),
                    op=ALU.mult,
                )

            # ================= main chunk loop =================
            po_ref = [None]

            def group_plan(ci):
                diag = {2 * ci, 2 * ci + 1}
                cand = [g for g in range(_NG) if g not in diag]
                nd = _DVE_N0 if ci == 0 else _DVE_N
                dve = set(cand[len(cand) - nd:]) if nd else set()
                return dve

            for ci in range(_NCH):
                dve_groups = group_plan(ci)
                po = pop.tile([_MT, _CH], F32, name="po", tag="po")
                po_ref[0] = po

                for g in range(_NG):
                    ps = psp.tile([_MT, 2 * _CH], F32, name="ps", tag="ps")
                    for j in range(2):
                        m = 2 * g + j
                        half = m % 2
                        rows = slice(_C * half, _C * (half + 1))
                        nc.tensor.matmul(
                            ps[:, _CH * j : _CH * (j + 1)],
                            lhsT=xr2[rows, _MT * m : _MT * (m + 1)],
                            rhs=xr2[rows, _CH * ci : _CH * (ci + 1)],
                            start=True, stop=True,
                            tile_position=(_C * half, 0),
                        )
                    if ci == 0:
                        emit_vt_groups(2 * (g + 1))
                    if False:
                        pass
                    else:
                        pt8 = p8p.tile([_MT, 2 * _CH], BF16, name="pt8",
                                       tag="pt8")
                        nc.scalar.activation(pt8[:], ps[:], AF.Exp,
                                             bias=ebias[:], scale=0.125)
                    for j in range(2):
                        t = 2 * g + j
                        nc.tensor.matmul(
                            po[0:_CP1, :],
                            lhsT=vt_block(vt8, t)[:, 0:_CP1],
                            rhs=pt8[:, _CH * j : _CH * (j + 1)],
                            start=(g == 0 and j == 0),
                            stop=(g == _NG - 1 and j == 1),
                            skip_group_check=True,
                        )
                # baseline-style inline tail (v1-proven ordering)
                usb, inv4 = emit_normalize(ci, po)
                emit_normalize2(ci, usb, inv4)
                if ci >= 1:
                    emit_dw(8 * ci - 1, 8 * ci)   # boundary row of chunk ci-1
                    emit_conv3(ci - 1)
                emit_dw(8 * ci, 8 * ci + 7)

            emit_dw(_N // _W - 1, _N // _W)
            emit_conv3(_NCH - 1)

    nc.finalize()
    return nc


def _get_nc():
    if "nc" not in _STATE:
        _STATE["nc"] = _build_program()
    return _STATE["nc"]


def _prep_inputs(x, w1, bn1_g, bn1_b, bn1_m, bn1_v,
                 w2, bn2_g, bn2_b, bn2_m, bn2_v,
                 w3, bn3_g, bn3_b, bn3_m, bn3_v):
    import ml_dtypes

    f32 = np.float32
    x = np.asarray(x, f32)
    inv1 = np.asarray(bn1_g, f32) / np.sqrt(np.asarray(bn1_v, f32) + _EPS)
    w1p = np.asarray(w1, f32)[:, :, 0, 0] * inv1[:, None]
    b1p = np.asarray(bn1_b, f32) - np.asarray(bn1_m, f32) * inv1
    w1aug = np.concatenate([w1p.T, b1p[None, :]], axis=0)

    inv2 = np.asarray(bn2_g, f32) / np.sqrt(np.asarray(bn2_v, f32) + _EPS)
    w2p = np.asarray(w2, f32)[:, 0].reshape(_C, 9) * inv2[:, None]
    b2p = (np.asarray(bn2_b, f32) - np.asarray(bn2_m, f32) * inv2)[:, None]

    inv3 = np.asarray(bn3_g, f32) / np.sqrt(np.asarray(bn3_v, f32) + _EPS)
    w3p = np.asarray(w3, f32)[:, :, 0, 0] * inv3[:, None]
    b3p = np.asarray(bn3_b, f32) - np.asarray(bn3_m, f32) * inv3
    w3aug = np.concatenate([w3p.T, b3p[None, :]], axis=0)

    consts = np.zeros((_CP1, _NCONST), f32)
    consts[:, 0:64] = w1aug
    consts[:, 64:128] = w3aug
    consts[0:_C, 128:129] = b2p
    for k in range(9):
        consts[0:_C, 129 + _C * k : 129 + _C * (k + 1)] = np.diag(w2p[:, k])

    consts8 = np.zeros((_C, _NCONST8), f32)
    for k in range(9):
        consts8[:, 64 * k : 64 * k + 64] = np.diag(w2p[:, k])
    consts8 = consts8.astype(ml_dtypes.float8_e4m3)

    B = x.shape[0]
    in_maps = []
    for i in range(B):
        in_maps.append({
            "x": np.ascontiguousarray(x[i].reshape(_C, _N)),
            "consts": consts,
            "consts8": consts8,
        })
    return in_maps


def kernel(**inputs) -> np.ndarray:
    from concourse.bass_utils import run_bass_kernel_spmd

    in_maps = _prep_inputs(**inputs)
    nc = _get_nc()
    _STATE["in_maps"] = in_maps
    res = run_bass_kernel_spmd(nc, in_maps, list(range(len(in_maps))))
    out = np.stack(
        [r["out"].reshape(_C, _W, _W) for r in res.results]
    ).astype(np.float32)
    return out


def profile_exec_time():
    """Re-run the last inputs with NTFF tracing; returns exec time in ns."""
    from concourse.bass_utils import run_bass_kernel_spmd

    nc = _get_nc()
    in_maps = _STATE.get("in_maps")
    assert in_maps is not None, "call kernel() first"
    res = run_bass_kernel_spmd(nc, in_maps, list(range(len(in_maps))),
                               trace=True)
    return res
